# revision 3
# baseline (speedup 1.0000x reference)
"""GCN layer v5: v3 + boundary-dup token packing.

Windows are padded only to the core-uniform max edge count (not to x128);
128-token tiles span window boundaries. A tile overlapping k windows gets k
matmuls, each with an rl column masked (999) to its window's tokens, so the
S one-hot zeroes foreign lanes. ~6% fewer gather descriptors than v3.
"""

import math
import sys

import numpy as np

for _p in ("/opt/trn_rl_repo",):
    if _p not in sys.path:
        sys.path.insert(0, _p)

import ml_dtypes  # noqa: E402

from concourse import bacc, bass, mybir, tile  # noqa: E402
from concourse import bass_utils  # noqa: E402
from concourse.bass import IndirectOffsetOnAxis  # noqa: E402

BF16 = mybir.dt.bfloat16
F32 = mybir.dt.float32
I32 = mybir.dt.int32
NP_BF16 = ml_dtypes.bfloat16

P = 128


def default_cfg():
    return dict(
        n_nodes=100000,
        n_edges=800000,
        in_f=128,
        out_f=64,
        n_cores=8,
        gw=7,
    )


def _derived(cfg):
    n_nodes = cfg["n_nodes"]
    c = cfg["n_cores"]
    ns = n_nodes // c
    nw = math.ceil(ns / P)
    npad = math.ceil(n_nodes / P) * P
    return ns, nw, npad


def prep_inputs(x, weights, bias, adj_rows, adj_cols, adj_vals, cfg):
    c = cfg["n_cores"]
    in_f, out_f, gw = cfg["in_f"], cfg["out_f"], cfg["gw"]
    ns, nw, npad = _derived(cfg)
    ngrp = math.ceil(nw / gw)

    x = np.asarray(x, dtype=np.float32)
    weights = np.asarray(weights, dtype=np.float32)
    bias = np.asarray(bias, dtype=np.float32)
    rows = np.asarray(adj_rows).astype(np.int64)
    cols = np.asarray(adj_cols).astype(np.int64)
    vals = np.asarray(adj_vals, dtype=np.float32)

    xb = np.zeros((npad, in_f), dtype=NP_BF16)
    xb[: x.shape[0]] = x.astype(NP_BF16)
    wt = weights.astype(NP_BF16)
    bias8 = np.tile(bias[None, :], (P, gw)).astype(np.float32)
    iota = np.ascontiguousarray(
        np.broadcast_to(np.arange(P, dtype=np.float32), (P, P))
    ).astype(NP_BF16)

    core = rows // ns
    w_full = (rows - core * ns) // P
    rloc = (rows - core * ns) % P

    cnt = np.bincount(core * nw + w_full, minlength=c * nw).reshape(c, nw)
    cnt_u = cnt.max(axis=0)  # core-uniform tokens per window

    # token offset of window w within its group's stream; group streams are
    # padded to x128 tiles
    grp_tiles = []  # tiles per group
    win_tok_off = np.zeros(nw, dtype=np.int64)  # token offset within group
    tile_base = np.zeros(ngrp + 1, dtype=np.int64)  # first tile of group
    for g in range(ngrp):
        w0, w1 = g * gw, min((g + 1) * gw, nw)
        pos = 0
        for w in range(w0, w1):
            win_tok_off[w] = pos
            pos += int(cnt_u[w])
        nt = -(-pos // P)
        grp_tiles.append(nt)
        tile_base[g + 1] = tile_base[g] + nt
    ntile = int(tile_base[-1])

    # static matmul list: per group, per tile, the windows it overlaps
    # mm list entries: (g, tile_in_group, w, start_flag, stop_flag, mm_col)
    mms = []  # (g, t_loc, w)
    for g in range(ngrp):
        w0, w1 = g * gw, min((g + 1) * gw, nw)
        for t in range(grp_tiles[g]):
            lo, hi = t * P, t * P + P
            for w in range(w0, w1):
                ws, we = win_tok_off[w], win_tok_off[w] + int(cnt_u[w])
                if ws < hi and we > lo:
                    mms.append((g, t, w))
    nmm = len(mms)
    # start/stop flags per window chain (program order == mms order)
    first_of_w = {}
    last_of_w = {}
    for i, (g, t, w) in enumerate(mms):
        if w not in first_of_w:
            first_of_w[w] = i
        last_of_w[w] = i

    # per-core tensors
    order = np.lexsort((cols, w_full, core))
    in_maps = []
    for ci in range(c):
        sel = order[core[order] == ci]
        w_c = w_full[sel]
        win_start = np.searchsorted(w_c, np.arange(nw))
        j = np.arange(len(sel)) - win_start[w_c]  # rank within window
        g_c = w_c // gw
        tok = tile_base[g_c] * P + win_tok_off[w_c] + j  # global token pos
        tcol = tok // P
        lane = tok % P

        gidx = np.zeros((P, ntile), dtype=np.int32)
        rl_tok = np.full((P, ntile), 999.0, dtype=np.float32)
        win_tok = np.full((P, ntile), -1, dtype=np.int64)
        vv = np.zeros((P, ntile), dtype=np.float32)
        gidx[lane, tcol] = cols[sel].astype(np.int32)
        rl_tok[lane, tcol] = rloc[sel].astype(np.float32)
        win_tok[lane, tcol] = w_c
        vv[lane, tcol] = vals[sel]

        # per-matmul rl columns: window-masked
        rlm = np.full((P, nmm), 999.0, dtype=np.float32)
        for i, (g, t, w) in enumerate(mms):
            tg = tile_base[g] + t
            m = win_tok[:, tg] == w
            rlm[m, i] = rl_tok[m, tg]

        in_maps.append(
            dict(xb=xb, wt=wt, bias8=bias8, iota=iota, gidx=gidx, rl=rlm, vv=vv)
        )

    meta = dict(
        grp_tiles=grp_tiles,
        tile_base=tile_base,
        mms=mms,
        first_of_w=first_of_w,
        last_of_w=last_of_w,
        ntile=ntile,
        nmm=nmm,
    )
    return in_maps, meta


def build(nc, meta, cfg):
    out_f, in_f, gw = cfg["out_f"], cfg["in_f"], cfg["gw"]
    ns, nw, npad = _derived(cfg)
    ngrp = math.ceil(nw / gw)
    grp_tiles = meta["grp_tiles"]
    tile_base = meta["tile_base"]
    mms = meta["mms"]
    first_of_w = meta["first_of_w"]
    last_of_w = meta["last_of_w"]
    ntile = meta["ntile"]
    nmm = meta["nmm"]
    assert in_f == P
    max_ntb = max(grp_tiles)

    xb_d = nc.dram_tensor("xb", [npad, in_f], BF16, kind="ExternalInput")
    wt_d = nc.dram_tensor("wt", [P, out_f], BF16, kind="ExternalInput")
    bias_d = nc.dram_tensor("bias8", [P, gw * out_f], F32, kind="ExternalInput")
    iota_d = nc.dram_tensor("iota", [P, P], BF16, kind="ExternalInput")
    gidx_d = nc.dram_tensor("gidx", [P, ntile], I32, kind="ExternalInput")
    rl_d = nc.dram_tensor("rl", [P, nmm], F32, kind="ExternalInput")
    vv_d = nc.dram_tensor("vv", [P, ntile], F32, kind="ExternalInput")
    out_d = nc.dram_tensor("out", [P, nw * out_f], F32, kind="ExternalOutput")

    eq = mybir.AluOpType.is_equal
    mul = mybir.AluOpType.mult
    add = mybir.AluOpType.add

    # group the matmul list by group for emission
    mm_by_g = [[] for _ in range(ngrp)]
    for i, (g, t, w) in enumerate(mms):
        mm_by_g[g].append((i, t, w))

    with tile.TileContext(nc) as tc:
        with (
            tc.tile_pool(name="const", bufs=1) as cpool,
            tc.tile_pool(name="gbuf", bufs=3) as gpool,
            tc.tile_pool(name="smat", bufs=8) as spool,
            tc.tile_pool(name="apsum", bufs=2, space="PSUM") as apspool,
            tc.tile_pool(name="aggsb", bufs=2) as aggpool,
            tc.tile_pool(name="ppsum", bufs=2, space="PSUM") as ppspool,
            tc.tile_pool(name="ot", bufs=2) as opool,
        ):
            wt_t = cpool.tile([P, out_f], BF16)
            nc.sync.dma_start(out=wt_t[:], in_=wt_d[:])
            iota_t = cpool.tile([P, P], BF16)
            nc.sync.dma_start(out=iota_t[:], in_=iota_d[:])
            bias_t = cpool.tile([P, gw * out_f], F32)
            nc.sync.dma_start(out=bias_t[:], in_=bias_d[:])
            idx_t = cpool.tile([P, ntile], I32)
            nc.scalar.dma_start(out=idx_t[:], in_=gidx_d[:])
            rl_t = cpool.tile([P, nmm], F32)
            nc.scalar.dma_start(out=rl_t[:], in_=rl_d[:])
            vv_t = cpool.tile([P, ntile], F32)
            nc.scalar.dma_start(out=vv_t[:], in_=vv_d[:])

            for g in range(ngrp):
                w0 = g * gw
                gwb = min(gw, nw - w0)
                t0 = int(tile_base[g])
                ntb = grp_tiles[g]

                gb = gpool.tile([P, max_ntb * in_f], BF16, tag="gb")
                for tb in range(ntb):
                    nc.gpsimd.indirect_dma_start(
                        out=gb[:, tb * in_f : (tb + 1) * in_f],
                        out_offset=None,
                        in_=xb_d[:],
                        in_offset=IndirectOffsetOnAxis(
                            ap=idx_t[:, t0 + tb : t0 + tb + 1], axis=0
                        ),
                    )

                agg_ps = apspool.tile([P, gw * P], F32, tag="agg")
                for i, t, w in mm_by_g[g]:
                    wl = w - w0
                    smat = spool.tile([P, P], BF16, tag="S")
                    nc.vector.tensor_scalar(
                        out=smat[:],
                        in0=iota_t[:],
                        scalar1=rl_t[:, i : i + 1],
                        scalar2=vv_t[:, t0 + t : t0 + t + 1],
                        op0=eq,
                        op1=mul,
                    )
                    nc.tensor.matmul(
                        out=agg_ps[:, wl * P : (wl + 1) * P],
                        lhsT=gb[:, t * in_f : (t + 1) * in_f],
                        rhs=smat[:],
                        start=(first_of_w[w] == i),
                        stop=(last_of_w[w] == i),
                    )

                agg_sb = aggpool.tile([P, gw * P], BF16, tag="aggsb")
                nc.vector.tensor_copy(
                    out=agg_sb[:, : gwb * P], in_=agg_ps[:, : gwb * P]
                )
                proj_ps = ppspool.tile([P, gw * out_f], F32, tag="proj")
                for wl in range(gwb):
                    nc.tensor.matmul(
                        out=proj_ps[:, wl * out_f : (wl + 1) * out_f],
                        lhsT=agg_sb[:, wl * P : (wl + 1) * P],
                        rhs=wt_t[:],
                        start=True,
                        stop=True,
                    )
                ot = opool.tile([P, gw * out_f], F32, tag="ot")
                nc.vector.tensor_tensor(
                    out=ot[:, : gwb * out_f],
                    in0=proj_ps[:, : gwb * out_f],
                    in1=bias_t[:, : gwb * out_f],
                    op=add,
                )
                nc.sync.dma_start(
                    out=out_d[:, w0 * out_f : (w0 + gwb) * out_f],
                    in_=ot[:, : gwb * out_f],
                )
    return nc


def assemble_output(results, cfg):
    out_f = cfg["out_f"]
    ns, nw, npad = _derived(cfg)
    blocks = []
    for r in results:
        o = np.asarray(r["out"], dtype=np.float32)
        o = o.reshape(P, nw, out_f).transpose(1, 0, 2).reshape(nw * P, out_f)[:ns]
        blocks.append(o)
    return np.ascontiguousarray(np.concatenate(blocks, axis=0))


LAST_RESULTS = None


def kernel(x, weights, bias, adj_rows, adj_cols, adj_vals):
    global LAST_RESULTS
    cfg = default_cfg()
    in_maps, meta = prep_inputs(x, weights, bias, adj_rows, adj_cols, adj_vals, cfg)
    nc = bacc.Bacc("TRN2", target_bir_lowering=False, debug=False)
    build(nc, meta, cfg)
    nc.compile()
    res = None
    for attempt in range(3):
        try:
            res = bass_utils.run_bass_kernel_spmd(
                nc, in_maps, core_ids=list(range(cfg["n_cores"]))
            )
            break
        except Exception:
            if attempt == 2:
                raise
    LAST_RESULTS = res
    return assemble_output(res.results, cfg)
